# revision 18
# baseline (speedup 1.0000x reference)
"""Trainium2 Bass kernel for nn_CaPa_MoE_clinical_MLP (CLAM-style dual-tower
attention-MIL with MoE head).

Strategy (8 NeuronCores, SPMD, no collectives):
  - Shard the N=30000 patch dimension: 3750 rows per core (padded to 3840
    on-device).
  - Per core: project h_virchow (2560->1024), attention nets for both
    modalities, raw attention scores A_raw [2, n_local] (kernel output), and
    the softmax-numerator partial  num = exp(A_raw).T @ hp  [2, 512] plus
    nothing else -- exp() without max-shift is safe because |A_raw| <~ 4.
  - Host: combine per-core partials (Z from the full A_raw output), then run
    the tiny expert/gate/fusion/classifier tail in fp32 numpy.

Matmuls run in bf16 with fp32 PSUM accumulation.  Layout trick: activations
are kept feature-major ("transposed", [f, n]); using a transposed activation
tile as matmul lhsT yields natural-layout outputs and vice versa, so only the
raw inputs need a real transpose, done via HWDGE xbar DMA-transpose (2-byte
dtype) from a bf16 DRAM staging copy produced by a casting SWDGE DMA.
"""

import numpy as np
import ml_dtypes

# ---- problem constants (hardcoded per task contract) ----
N_CORES = 8
N_TOTAL = 30000
NSH = N_TOTAL // N_CORES        # 3750 rows per core
NPAD = 3840                     # 30 * 128
VIR, EMB, H, D, NCLS, CLIN = 2560, 1024, 512, 256, 2, 6
TAU, EPS = 1.0, 1e-20

# jax.random.uniform(jax.random.key(42), (1, 3)) -- fixed in the reference.
# Filled in from the cached reference; verified by test.py.
GUMBEL_U = np.array([[0.59400654, 0.43801308, 0.6285691]], dtype=np.float32)

BF16 = ml_dtypes.bfloat16

_CACHE = {}


def _blocks():
    """(n0, nb, valid) DMA/compute blocks covering NPAD rows."""
    out = []
    n0 = 0
    while n0 < NPAD:
        nb = min(512, NPAD - n0)
        out.append((n0, nb, max(0, min(NSH - n0, nb))))
        n0 += nb
    return out


def _build_bass(reps=1):
    import concourse.mybir as mybir
    from concourse import bacc
    from concourse.tile import TileContext
    from concourse.masks import make_identity

    fp32 = mybir.dt.float32
    bf16 = mybir.dt.bfloat16
    AF = mybir.ActivationFunctionType

    nc = bacc.Bacc("TRN2", target_bir_lowering=False)

    # ---------------- I/O ----------------
    # bf16, host-padded to NPAD rows (zeros) so xbar transpose tiling is clean
    hv = nc.dram_tensor("hv16", [NPAD, VIR], bf16, kind="ExternalInput")
    hu = nc.dram_tensor("hu16", [NPAD, EMB], bf16, kind="ExternalInput")

    def win(name, shape, dt=bf16):
        return nc.dram_tensor(name, shape, dt, kind="ExternalInput")

    wp = win("wp16", [VIR, EMB])
    bp = win("bp", [128, EMB // 128], fp32)
    wio = {}
    for m in ("v", "u"):
        wio[f"wfc_{m}"] = win(f"wfc_{m}16", [EMB, H])
        wio[f"bfc_{m}"] = win(f"bfc_{m}", [128, H // 128], fp32)
        wio[f"wa_{m}"] = win(f"wa_{m}16", [H, D])
        wio[f"ba_{m}"] = win(f"ba_{m}", [128, D // 128], fp32)
        wio[f"wb_{m}"] = win(f"wb_{m}16", [H, D])
        wio[f"bb_{m}"] = win(f"bb_{m}", [128, D // 128], fp32)
        wio[f"wc_{m}"] = win(f"wc_{m}16", [D, NCLS])
        wio[f"bc_{m}"] = win(f"bc_{m}", [NCLS, 1], fp32)
        wio[f"bcrow_{m}"] = win(f"bcrow_{m}16", [1, NCLS])

    av_out = nc.dram_tensor("av_out", [NCLS, NSH], fp32, kind="ExternalOutput")
    au_out = nc.dram_tensor("au_out", [NCLS, NSH], fp32, kind="ExternalOutput")
    num_v = nc.dram_tensor("num_v", [NCLS, H], fp32, kind="ExternalOutput")
    num_u = nc.dram_tensor("num_u", [NCLS, H], fp32, kind="ExternalOutput")

    blocks = _blocks()

    with TileContext(nc) as tc:
        with (
            tc.tile_pool(name="consts", bufs=1) as consts,
            tc.tile_pool(name="wpool", bufs=1) as wpool,
            tc.tile_pool(name="xin", bufs=26) as xinp,
            tc.tile_pool(name="work", bufs=2) as work,
            tc.tile_pool(name="hpn", bufs=4) as hpnp,
            tc.tile_pool(name="small", bufs=1) as small,
            tc.tile_pool(name="mmps", bufs=3, space="PSUM") as mmps,
            tc.tile_pool(name="tpps", bufs=2, space="PSUM") as tpps,
            tc.tile_pool(name="pnps", bufs=1, space="PSUM") as pnps,
            tc.tile_pool(name="plps", bufs=1, space="PSUM") as plps,
        ):
            # ---------------- constants / weights to SBUF ----------------
            ident = consts.tile([128, 128], bf16)
            make_identity(nc, ident)
            ones_row = consts.tile([1, 128], bf16)
            nc.vector.memset(ones_row, 1.0)

            wp_sb = wpool.tile([128, VIR // 128, EMB], bf16)
            nc.gpsimd.dma_start(wp_sb, wp[:, :].rearrange("(kt p) f -> p kt f", p=128))
            bp_sb = wpool.tile([128, EMB // 128], fp32)
            nc.gpsimd.dma_start(bp_sb, bp[:, :])

            wsb = {}
            for m in ("v", "u"):
                for key, kdim, fdim in (
                    (f"wfc_{m}", EMB, H),
                    (f"wa_{m}", H, D),
                    (f"wb_{m}", H, D),
                    (f"wc_{m}", D, NCLS),
                ):
                    t = wpool.tile([128, kdim // 128, fdim], bf16, name=f"sb_{key}")
                    nc.gpsimd.dma_start(
                        t, wio[key][:, :].rearrange("(kt p) f -> p kt f", p=128)
                    )
                    wsb[key] = t
                for key in (f"bfc_{m}", f"ba_{m}", f"bb_{m}", f"bc_{m}"):
                    t = wpool.tile(list(wio[key].shape), fp32, name=f"sb_{key}")
                    nc.gpsimd.dma_start(t, wio[key][:, :])
                    wsb[key] = t
                t = wpool.tile([1, NCLS], bf16, name=f"sb_bcrow_{m}")
                nc.gpsimd.dma_start(t, wio[f"bcrow_{m}"][:, :])
                wsb[f"bcrow_{m}"] = t

            h16 = {"v": hv, "u": hu}

            # ---------------- per-modality tower ----------------
            def tower(m, kin_tiles, has_proj, a_out, num_out):
                w_fc = wsb[f"wfc_{m}"]
                b_fc = wsb[f"bfc_{m}"]
                w_a, b_a = wsb[f"wa_{m}"], wsb[f"ba_{m}"]
                w_b, b_b = wsb[f"wb_{m}"], wsb[f"bb_{m}"]
                w_c, b_c = wsb[f"wc_{m}"], wsb[f"bc_{m}"]
                bcrow = wsb[f"bcrow_{m}"]

                pool_ps = plps.tile([NCLS, H], fp32, name=f"pool_{m}", tag=f"pool_{m}")
                n_pool_mm = sum(nb // 128 for _, nb, _ in blocks)
                pool_i = 0

                for b, (n0, nb, valid) in enumerate(blocks):
                    src = h16[m]
                    # transposed input tiles [128k, nb]
                    xin = []
                    for kt in range(kin_tiles):
                        t = xinp.tile([128, nb], bf16, name=f"xin_{m}{b}_{kt}", tag="xin")
                        nc.sync.dma_start_transpose(
                            t, src[n0 : n0 + nb, kt * 128 : (kt + 1) * 128]
                        )
                        xin.append(t)

                    if has_proj:
                        x16 = work.tile(
                            [128, EMB // 128, nb], bf16, name=f"x16_{b}", tag="x16"
                        )
                        for ft in range(EMB // 128):
                            ps = mmps.tile(
                                [128, 512], fp32, name=f"ps_p{b}_{ft}", tag="mm"
                            )[:, :nb]
                            for kt in range(kin_tiles):
                                nc.tensor.matmul(
                                    ps,
                                    wp_sb[:, kt, ft * 128 : (ft + 1) * 128],
                                    xin[kt],
                                    start=(kt == 0),
                                    stop=(kt == kin_tiles - 1),
                                )
                            nc.scalar.activation(
                                x16[:, ft, :], ps, AF.Identity,
                                bias=bp_sb[:, ft : ft + 1],
                            )
                        feat = [x16[:, j, :] for j in range(EMB // 128)]
                    else:
                        feat = xin

                    nfeat = len(feat)
                    # fc -> hpt [f', n] (relu, transposed layout)
                    hpt = work.tile(
                        [128, H // 128, nb], bf16, name=f"hpt_{m}{b}", tag="hpt"
                    )
                    for ft in range(H // 128):
                        ps = mmps.tile(
                            [128, 512], fp32, name=f"ps_fc{b}_{ft}", tag="mm"
                        )[:, :nb]
                        for kt in range(nfeat):
                            nc.tensor.matmul(
                                ps,
                                w_fc[:, kt, ft * 128 : (ft + 1) * 128],
                                feat[kt],
                                start=(kt == 0),
                                stop=(kt == nfeat - 1),
                            )
                        nc.scalar.activation(
                            hpt[:, ft, :], ps, AF.Relu, bias=b_fc[:, ft : ft + 1]
                        )

                    # gated attention: aT = tanh(.), bT = sigmoid(.), gT = aT*bT
                    gt = work.tile(
                        [128, D // 128, nb], bf16, name=f"gt_{m}{b}", tag="gt"
                    )
                    at = work.tile(
                        [128, D // 128, nb], bf16, name=f"at_{m}{b}", tag="at"
                    )
                    bt = work.tile(
                        [128, D // 128, nb], bf16, name=f"bt_{m}{b}", tag="bt"
                    )
                    for ft in range(D // 128):
                        psa = mmps.tile(
                            [128, 512], fp32, name=f"ps_a{b}_{ft}", tag="mm"
                        )[:, :nb]
                        for kt in range(H // 128):
                            nc.tensor.matmul(
                                psa,
                                w_a[:, kt, ft * 128 : (ft + 1) * 128],
                                hpt[:, kt, :],
                                start=(kt == 0),
                                stop=(kt == H // 128 - 1),
                            )
                        nc.scalar.activation(
                            at[:, ft, :], psa, AF.Tanh, bias=b_a[:, ft : ft + 1]
                        )
                        psb = mmps.tile(
                            [128, 512], fp32, name=f"ps_b{b}_{ft}", tag="mm"
                        )[:, :nb]
                        for kt in range(H // 128):
                            nc.tensor.matmul(
                                psb,
                                w_b[:, kt, ft * 128 : (ft + 1) * 128],
                                hpt[:, kt, :],
                                start=(kt == 0),
                                stop=(kt == H // 128 - 1),
                            )
                        nc.scalar.activation(
                            bt[:, ft, :], psb, AF.Sigmoid, bias=b_b[:, ft : ft + 1]
                        )
                        nc.vector.tensor_mul(
                            out=gt[:, ft, :], in0=at[:, ft, :], in1=bt[:, ft, :]
                        )

                    # raw attention scores, transposed layout [2, nb] (output)
                    psr = mmps.tile([128, 512], fp32, name=f"ps_r{b}", tag="mm")[
                        :NCLS, :nb
                    ]
                    for kt in range(D // 128):
                        nc.tensor.matmul(
                            psr,
                            w_c[:, kt, :],
                            gt[:, kt, :],
                            start=(kt == 0),
                            stop=(kt == D // 128 - 1),
                        )
                    if valid > 0:
                        araw = work.tile(
                            [NCLS, 512], fp32, name=f"araw_{m}{b}", tag="araw"
                        )[:, :nb]
                        nc.scalar.activation(araw, psr, AF.Identity, bias=b_c)
                        nc.gpsimd.dma_start(
                            a_out[:, n0 : n0 + valid], araw[:, :valid]
                        )

                    # pooling: per 128-row subtile, natural-layout exp weights
                    # via matmul (lhsT = transposed activations trick), then
                    # num += E_nat.T @ HP_nat
                    for s in range(nb // 128):
                        sl = slice(s * 128, (s + 1) * 128)
                        # araw in natural layout [128n, 2]
                        psn = pnps.tile(
                            [128, NCLS], fp32, name=f"psn_{m}{b}_{s}", tag="pnat"
                        )
                        for kt in range(D // 128):
                            nc.tensor.matmul(
                                psn,
                                gt[:, kt, sl],
                                w_c[:, kt, :],
                                start=(kt == 0),
                                stop=False,
                            )
                        # fold per-class bias: += ones.T @ bcrow
                        nc.tensor.matmul(
                            psn, ones_row, bcrow, start=False, stop=True
                        )
                        e_nat = hpnp.tile(
                            [128, NCLS], bf16, name=f"enat_{m}{b}_{s}", tag="enat"
                        )
                        sub_valid = max(0, min(valid - s * 128, 128))
                        if sub_valid < 128:
                            # zero first, then exp() only the valid rows
                            # (partition offsets must be 32-aligned, so we
                            # cannot memset [sub_valid:, :] directly)
                            nc.vector.memset(e_nat, 0.0)
                            nc.scalar.activation(
                                e_nat[:sub_valid, :], psn[:sub_valid, :], AF.Exp
                            )
                        else:
                            nc.scalar.activation(e_nat, psn, AF.Exp)

                        # HP natural layout via PE transpose of hpt
                        hpn = hpnp.tile(
                            [128, H], bf16, name=f"hpn_{m}{b}_{s}", tag="hpn"
                        )
                        for ft in range(H // 128):
                            pst = tpps.tile(
                                [128, 128], bf16, name=f"pst_{m}{b}_{s}_{ft}",
                                tag="tp",
                            )
                            nc.tensor.transpose(pst, hpt[:, ft, sl], ident)
                            nc.vector.tensor_copy(
                                out=hpn[:, ft * 128 : (ft + 1) * 128], in_=pst
                            )

                        nc.tensor.matmul(
                            pool_ps,
                            e_nat,
                            hpn,
                            start=(pool_i == 0),
                            stop=(pool_i == n_pool_mm - 1),
                        )
                        pool_i += 1

                num_sb = small.tile([NCLS, H], fp32, name=f"num_sb_{m}", tag=f"num_{m}")
                nc.vector.tensor_copy(out=num_sb, in_=pool_ps)
                nc.gpsimd.dma_start(num_out[:, :], num_sb)

            def body():
                with nc.named_scope("tower_v"):
                    tower("v", VIR // 128, True, av_out, num_v)
                with nc.named_scope("tower_u"):
                    tower("u", EMB // 128, False, au_out, num_u)

            if reps == 1:
                body()
            else:
                # benchmarking variant: run the body `reps` times in a HW loop
                with tc.For_i(0, reps, 1):
                    body()

    nc.finalize()
    return nc


def _get_nc(reps=1):
    key = ("nc", reps)
    if key not in _CACHE:
        _CACHE[key] = _build_bass(reps)
    return _CACHE[key]


def _prep_weights(params):
    def a32(x):
        return np.asarray(x, dtype=np.float32)

    def tile_bias(b, ntiles):
        return np.ascontiguousarray(a32(b).reshape(ntiles, 128).T)

    sh = {
        "wp16": a32(params["proj_v"]["w"]).astype(BF16),
        "bp": tile_bias(params["proj_v"]["b"], EMB // 128),
    }
    for m, att in (("v", params["attn_v"]), ("u", params["attn_u"])):
        sh[f"wfc_{m}16"] = a32(att["fc"]["w"]).astype(BF16)
        sh[f"bfc_{m}"] = tile_bias(att["fc"]["b"], H // 128)
        sh[f"wa_{m}16"] = a32(att["a"]["w"]).astype(BF16)
        sh[f"ba_{m}"] = tile_bias(att["a"]["b"], D // 128)
        sh[f"wb_{m}16"] = a32(att["b"]["w"]).astype(BF16)
        sh[f"bb_{m}"] = tile_bias(att["b"]["b"], D // 128)
        sh[f"wc_{m}16"] = a32(att["c"]["w"]).astype(BF16)
        sh[f"bc_{m}"] = np.ascontiguousarray(a32(att["c"]["b"]).reshape(NCLS, 1))
        sh[f"bcrow_{m}16"] = a32(att["c"]["b"]).reshape(1, NCLS).astype(BF16)
    return sh


def _run_device(h_virchow, h_UNI, params):
    from concourse.bass_utils import run_bass_kernel_spmd

    nc = _get_nc()
    shared = _prep_weights(params)
    in_maps = []
    for c in range(N_CORES):
        m = dict(shared)
        hv16 = np.zeros((NPAD, VIR), dtype=BF16)
        np.copyto(hv16[:NSH], h_virchow[c * NSH : (c + 1) * NSH], casting="unsafe")
        hu16 = np.zeros((NPAD, EMB), dtype=BF16)
        np.copyto(hu16[:NSH], h_UNI[c * NSH : (c + 1) * NSH], casting="unsafe")
        m["hv16"] = hv16
        m["hu16"] = hu16
        in_maps.append(m)

    res = run_bass_kernel_spmd(nc, in_maps, core_ids=list(range(N_CORES)))
    _CACHE["last_results"] = res
    return res.results


def _softmax(x, axis):
    x = x - np.max(x, axis=axis, keepdims=True)
    e = np.exp(x)
    return e / np.sum(e, axis=axis, keepdims=True)


def _host_tail(M_v, M_u, clinical, params):
    """Mirror of the reference MoE/gate/fusion/classifier tail in fp32 numpy."""

    def a32(x):
        return np.asarray(x, dtype=np.float32)

    def lin(p, x):
        return x @ a32(p["w"]) + a32(p["b"])

    def relu(x):
        return np.maximum(x, np.float32(0.0))

    def expert(p, x):
        return relu(lin(p["l2"], relu(lin(p["l1"], x))))

    c = relu(lin(params["clin"], a32(clinical)))
    g = np.concatenate([M_v.mean(0), M_u.mean(0)])
    gate_in = np.concatenate([g, c]).astype(np.float32)
    gate_logits = lin(
        params["gate"]["l2"], relu(lin(params["gate"]["l1"], gate_in))
    )[None, :]

    gum = -np.log(-np.log(GUMBEL_U + np.float32(EPS)) + np.float32(EPS)).astype(
        np.float32
    )
    y = _softmax((gate_logits + gum) / np.float32(TAU), axis=-1).astype(np.float32)
    idx = int(np.argmax(y, axis=-1)[0])
    y_hard = np.zeros_like(y)
    y_hard[0, idx] = 1.0
    w = ((y_hard - y) + y)[0]

    E1 = expert(params["e1"], M_v)
    E3 = expert(params["e3"], M_u)
    E2 = lin(
        params["e2_out"],
        expert(params["e2"], np.concatenate([M_v, M_u], axis=-1)),
    )
    M_moe = w[0] * E1 + w[1] * E2 + w[2] * E3

    c_b = np.broadcast_to(c, M_moe.shape)
    M_fused = lin(params["fusion"], np.concatenate([M_moe, c_b], axis=-1))

    cls_w = a32(params["cls"]["w"])
    cls_b = a32(params["cls"]["b"])
    logits = (np.sum(M_fused * cls_w, axis=-1) + cls_b)[None, :].astype(np.float32)
    Y_prob = _softmax(logits, axis=1).astype(np.float32)
    Y_hat = np.argmax(logits, axis=1).astype(np.int32)
    return logits, Y_prob, Y_hat


def kernel(h_virchow, h_UNI, clinical, params):
    h_virchow = np.asarray(h_virchow, dtype=np.float32)
    h_UNI = np.asarray(h_UNI, dtype=np.float32)
    clinical = np.asarray(clinical, dtype=np.float32)

    results = _run_device(h_virchow, h_UNI, params)

    A_v = np.concatenate([r["av_out"] for r in results], axis=1)
    A_u = np.concatenate([r["au_out"] for r in results], axis=1)

    def pooled(a_full, key):
        num = np.zeros((NCLS, H), dtype=np.float64)
        for r in results:
            num += r[key].astype(np.float64)
        Z = np.exp(a_full.astype(np.float64)).sum(axis=1)
        return (num / Z[:, None]).astype(np.float32)

    M_v = pooled(A_v, "num_v")
    M_u = pooled(A_u, "num_u")

    logits, Y_prob, Y_hat = _host_tail(M_v, M_u, clinical, params)
    return logits, Y_prob, Y_hat, A_v, A_u


# revision 24
# speedup vs baseline: 1.3919x; 1.3919x over previous
"""Trainium2 Bass kernel for nn_CaPa_MoE_clinical_MLP (CLAM-style dual-tower
attention-MIL with MoE head).

Strategy (8 NeuronCores, SPMD, no collectives):
  - Shard the N=30000 patch dimension: 3750 rows per core (padded to 3840
    on-device).
  - Per core: project h_virchow (2560->1024), attention nets for both
    modalities, raw attention scores A_raw [2, n_local] (kernel output), and
    the softmax-numerator partial  num = exp(A_raw).T @ hp  [2, 512] plus
    nothing else -- exp() without max-shift is safe because |A_raw| <~ 4.
  - Host: combine per-core partials (Z from the full A_raw output), then run
    the tiny expert/gate/fusion/classifier tail in fp32 numpy.

Matmuls run in bf16 with fp32 PSUM accumulation.  Layout trick: activations
are kept feature-major ("transposed", [f, n]); using a transposed activation
tile as matmul lhsT yields natural-layout outputs and vice versa, so only the
raw inputs need a real transpose, done via HWDGE xbar DMA-transpose (2-byte
dtype) from a bf16 DRAM staging copy produced by a casting SWDGE DMA.
"""

import numpy as np
import ml_dtypes

# ---- problem constants (hardcoded per task contract) ----
N_CORES = 8
N_TOTAL = 30000
NSH = N_TOTAL // N_CORES        # 3750 rows per core
NPAD = 3840                     # 30 * 128
VIR, EMB, H, D, NCLS, CLIN = 2560, 1024, 512, 256, 2, 6
TAU, EPS = 1.0, 1e-20

# jax.random.uniform(jax.random.key(42), (1, 3)) -- fixed in the reference.
# Filled in from the cached reference; verified by test.py.
GUMBEL_U = np.array([[0.59400654, 0.43801308, 0.6285691]], dtype=np.float32)

BF16 = ml_dtypes.bfloat16

_CACHE = {}


def _blocks():
    """(n0, nb, valid) DMA/compute blocks covering NPAD rows."""
    out = []
    n0 = 0
    while n0 < NPAD:
        nb = min(512, NPAD - n0)
        out.append((n0, nb, max(0, min(NSH - n0, nb))))
        n0 += nb
    return out


def _build_bass(reps=1, opts=None):
    import concourse.mybir as mybir
    from concourse import bacc
    from concourse.tile import TileContext
    from concourse.masks import make_identity

    fp32 = mybir.dt.float32
    bf16 = mybir.dt.bfloat16
    AF = mybir.ActivationFunctionType

    opts = dict(opts or {})
    xin_bufs = opts.get("xin_bufs", 26)
    work_bufs = opts.get("work_bufs", 2)
    split_xpose = opts.get("split_xpose", False)   # alternate SP/ACT rings
    ablate = opts.get("ablate", None)              # None | "noxpose" | "xonly"

    nc = bacc.Bacc("TRN2", target_bir_lowering=False)

    # ---------------- I/O ----------------
    # bf16, host-padded to NPAD rows (zeros) so xbar transpose tiling is clean
    hv = nc.dram_tensor("hv16", [NPAD, VIR], bf16, kind="ExternalInput")
    hu = nc.dram_tensor("hu16", [NPAD, EMB], bf16, kind="ExternalInput")

    def win(name, shape, dt=bf16):
        return nc.dram_tensor(name, shape, dt, kind="ExternalInput")

    wp = win("wp16", [VIR, EMB])
    bp = win("bp", [128, EMB // 128], fp32)
    wio = {}
    for m in ("v", "u"):
        wio[f"wfc_{m}"] = win(f"wfc_{m}16", [EMB, H])
        wio[f"bfc_{m}"] = win(f"bfc_{m}", [128, H // 128], fp32)
        wio[f"wa_{m}"] = win(f"wa_{m}16", [H, D])
        wio[f"ba_{m}"] = win(f"ba_{m}", [128, D // 128], fp32)
        wio[f"wb_{m}"] = win(f"wb_{m}16", [H, D])
        wio[f"bb_{m}"] = win(f"bb_{m}", [128, D // 128], fp32)
        wio[f"wc_{m}"] = win(f"wc_{m}16", [D, NCLS])
        wio[f"bc_{m}"] = win(f"bc_{m}", [NCLS, 1], fp32)
        wio[f"bcrow_{m}"] = win(f"bcrow_{m}16", [1, NCLS])

    av_out = nc.dram_tensor("av_out", [NCLS, NSH], fp32, kind="ExternalOutput")
    au_out = nc.dram_tensor("au_out", [NCLS, NSH], fp32, kind="ExternalOutput")
    num_v = nc.dram_tensor("num_v", [NCLS, H], fp32, kind="ExternalOutput")
    num_u = nc.dram_tensor("num_u", [NCLS, H], fp32, kind="ExternalOutput")

    blocks = _blocks()

    with TileContext(nc) as tc:
        with (
            tc.tile_pool(name="consts", bufs=1) as consts,
            tc.tile_pool(name="wpool", bufs=1) as wpool,
            tc.tile_pool(name="xin", bufs=xin_bufs) as xinp,
            tc.tile_pool(name="work", bufs=work_bufs) as work,
            tc.tile_pool(name="hpn", bufs=4) as hpnp,
            tc.tile_pool(name="small", bufs=1) as small,
            tc.tile_pool(name="mmps", bufs=3, space="PSUM") as mmps,
            tc.tile_pool(name="tpps", bufs=2, space="PSUM") as tpps,
            tc.tile_pool(name="pnps", bufs=1, space="PSUM") as pnps,
            tc.tile_pool(name="plps", bufs=1, space="PSUM") as plps,
        ):
            # ---------------- constants / weights to SBUF ----------------
            ident = consts.tile([128, 128], bf16)
            make_identity(nc, ident)
            ones_row = consts.tile([1, 128], bf16)
            nc.vector.memset(ones_row, 1.0)

            wp_sb = wpool.tile([128, VIR // 128, EMB], bf16)
            nc.gpsimd.dma_start(wp_sb, wp[:, :].rearrange("(kt p) f -> p kt f", p=128))
            bp_sb = wpool.tile([128, EMB // 128], fp32)
            nc.gpsimd.dma_start(bp_sb, bp[:, :])

            wsb = {}
            for m in ("v", "u"):
                for key, kdim, fdim in (
                    (f"wfc_{m}", EMB, H),
                    (f"wa_{m}", H, D),
                    (f"wb_{m}", H, D),
                    (f"wc_{m}", D, NCLS),
                ):
                    t = wpool.tile([128, kdim // 128, fdim], bf16, name=f"sb_{key}")
                    nc.gpsimd.dma_start(
                        t, wio[key][:, :].rearrange("(kt p) f -> p kt f", p=128)
                    )
                    wsb[key] = t
                for key in (f"bfc_{m}", f"ba_{m}", f"bb_{m}", f"bc_{m}"):
                    t = wpool.tile(list(wio[key].shape), fp32, name=f"sb_{key}")
                    nc.gpsimd.dma_start(t, wio[key][:, :])
                    wsb[key] = t
                t = wpool.tile([1, NCLS], bf16, name=f"sb_bcrow_{m}")
                nc.gpsimd.dma_start(t, wio[f"bcrow_{m}"][:, :])
                wsb[f"bcrow_{m}"] = t

            h16 = {"v": hv, "u": hu}

            # ---------------- per-modality tower ----------------
            def tower(m, kin_tiles, has_proj, a_out, num_out):
                w_fc = wsb[f"wfc_{m}"]
                b_fc = wsb[f"bfc_{m}"]
                w_a, b_a = wsb[f"wa_{m}"], wsb[f"ba_{m}"]
                w_b, b_b = wsb[f"wb_{m}"], wsb[f"bb_{m}"]
                w_c, b_c = wsb[f"wc_{m}"], wsb[f"bc_{m}"]
                bcrow = wsb[f"bcrow_{m}"]

                pool_ps = plps.tile([NCLS, H], fp32, name=f"pool_{m}", tag=f"pool_{m}")
                n_pool_mm = sum(nb // 128 for _, nb, _ in blocks)
                pool_i = 0

                for b, (n0, nb, valid) in enumerate(blocks):
                    src = h16[m]
                    # transposed input tiles [128k, nb]
                    xin = []
                    for kt in range(kin_tiles):
                        t = xinp.tile([128, nb], bf16, name=f"xin_{m}{b}_{kt}", tag="xin")
                        if ablate == "noxpose":
                            nc.vector.memset(t, 0.001)
                        else:
                            eng = nc.scalar if (split_xpose and kt % 2) else nc.sync
                            eng.dma_start_transpose(
                                t, src[n0 : n0 + nb, kt * 128 : (kt + 1) * 128]
                            )
                        xin.append(t)
                    if ablate == "xonly":
                        continue

                    if has_proj:
                        x16 = work.tile(
                            [128, EMB // 128, nb], bf16, name=f"x16_{b}", tag="x16"
                        )
                        for ft in range(EMB // 128):
                            ps = mmps.tile(
                                [128, 512], fp32, name=f"ps_p{b}_{ft}", tag="mm"
                            )[:, :nb]
                            for kt in range(kin_tiles):
                                nc.tensor.matmul(
                                    ps,
                                    wp_sb[:, kt, ft * 128 : (ft + 1) * 128],
                                    xin[kt],
                                    start=(kt == 0),
                                    stop=(kt == kin_tiles - 1),
                                )
                            nc.scalar.activation(
                                x16[:, ft, :], ps, AF.Identity,
                                bias=bp_sb[:, ft : ft + 1],
                            )
                        feat = [x16[:, j, :] for j in range(EMB // 128)]
                    else:
                        feat = xin

                    nfeat = len(feat)
                    # fc -> hpt [f', n] (relu, transposed layout)
                    hpt = work.tile(
                        [128, H // 128, nb], bf16, name=f"hpt_{m}{b}", tag="hpt"
                    )
                    for ft in range(H // 128):
                        ps = mmps.tile(
                            [128, 512], fp32, name=f"ps_fc{b}_{ft}", tag="mm"
                        )[:, :nb]
                        for kt in range(nfeat):
                            nc.tensor.matmul(
                                ps,
                                w_fc[:, kt, ft * 128 : (ft + 1) * 128],
                                feat[kt],
                                start=(kt == 0),
                                stop=(kt == nfeat - 1),
                            )
                        nc.scalar.activation(
                            hpt[:, ft, :], ps, AF.Relu, bias=b_fc[:, ft : ft + 1]
                        )

                    # gated attention: aT = tanh(.), bT = sigmoid(.), gT = aT*bT
                    gt = work.tile(
                        [128, D // 128, nb], bf16, name=f"gt_{m}{b}", tag="gt"
                    )
                    at = work.tile(
                        [128, D // 128, nb], bf16, name=f"at_{m}{b}", tag="at"
                    )
                    bt = work.tile(
                        [128, D // 128, nb], bf16, name=f"bt_{m}{b}", tag="bt"
                    )
                    for ft in range(D // 128):
                        psa = mmps.tile(
                            [128, 512], fp32, name=f"ps_a{b}_{ft}", tag="mm"
                        )[:, :nb]
                        for kt in range(H // 128):
                            nc.tensor.matmul(
                                psa,
                                w_a[:, kt, ft * 128 : (ft + 1) * 128],
                                hpt[:, kt, :],
                                start=(kt == 0),
                                stop=(kt == H // 128 - 1),
                            )
                        nc.scalar.activation(
                            at[:, ft, :], psa, AF.Tanh, bias=b_a[:, ft : ft + 1]
                        )
                        psb = mmps.tile(
                            [128, 512], fp32, name=f"ps_b{b}_{ft}", tag="mm"
                        )[:, :nb]
                        for kt in range(H // 128):
                            nc.tensor.matmul(
                                psb,
                                w_b[:, kt, ft * 128 : (ft + 1) * 128],
                                hpt[:, kt, :],
                                start=(kt == 0),
                                stop=(kt == H // 128 - 1),
                            )
                        nc.scalar.activation(
                            bt[:, ft, :], psb, AF.Sigmoid, bias=b_b[:, ft : ft + 1]
                        )
                        nc.vector.tensor_mul(
                            out=gt[:, ft, :], in0=at[:, ft, :], in1=bt[:, ft, :]
                        )

                    # raw attention scores, transposed layout [2, nb] (output)
                    psr = mmps.tile([128, 512], fp32, name=f"ps_r{b}", tag="mm")[
                        :NCLS, :nb
                    ]
                    for kt in range(D // 128):
                        nc.tensor.matmul(
                            psr,
                            w_c[:, kt, :],
                            gt[:, kt, :],
                            start=(kt == 0),
                            stop=(kt == D // 128 - 1),
                        )
                    if valid > 0:
                        araw = work.tile(
                            [NCLS, 512], fp32, name=f"araw_{m}{b}", tag="araw"
                        )[:, :nb]
                        nc.scalar.activation(araw, psr, AF.Identity, bias=b_c)
                        nc.gpsimd.dma_start(
                            a_out[:, n0 : n0 + valid], araw[:, :valid]
                        )

                    # pooling: per 128-row subtile, natural-layout exp weights
                    # via matmul (lhsT = transposed activations trick), then
                    # num += E_nat.T @ HP_nat
                    for s in range(nb // 128):
                        sl = slice(s * 128, (s + 1) * 128)
                        # araw in natural layout [128n, 2]
                        psn = pnps.tile(
                            [128, NCLS], fp32, name=f"psn_{m}{b}_{s}", tag="pnat"
                        )
                        for kt in range(D // 128):
                            nc.tensor.matmul(
                                psn,
                                gt[:, kt, sl],
                                w_c[:, kt, :],
                                start=(kt == 0),
                                stop=False,
                            )
                        # fold per-class bias: += ones.T @ bcrow
                        nc.tensor.matmul(
                            psn, ones_row, bcrow, start=False, stop=True
                        )
                        e_nat = hpnp.tile(
                            [128, NCLS], bf16, name=f"enat_{m}{b}_{s}", tag="enat"
                        )
                        sub_valid = max(0, min(valid - s * 128, 128))
                        if sub_valid < 128:
                            # zero first, then exp() only the valid rows
                            # (partition offsets must be 32-aligned, so we
                            # cannot memset [sub_valid:, :] directly)
                            nc.vector.memset(e_nat, 0.0)
                            nc.scalar.activation(
                                e_nat[:sub_valid, :], psn[:sub_valid, :], AF.Exp
                            )
                        else:
                            nc.scalar.activation(e_nat, psn, AF.Exp)

                        # HP natural layout via PE transpose of hpt
                        hpn = hpnp.tile(
                            [128, H], bf16, name=f"hpn_{m}{b}_{s}", tag="hpn"
                        )
                        for ft in range(H // 128):
                            pst = tpps.tile(
                                [128, 128], bf16, name=f"pst_{m}{b}_{s}_{ft}",
                                tag="tp",
                            )
                            nc.tensor.transpose(pst, hpt[:, ft, sl], ident)
                            nc.vector.tensor_copy(
                                out=hpn[:, ft * 128 : (ft + 1) * 128], in_=pst
                            )

                        nc.tensor.matmul(
                            pool_ps,
                            e_nat,
                            hpn,
                            start=(pool_i == 0),
                            stop=(pool_i == n_pool_mm - 1),
                        )
                        pool_i += 1

                if ablate != "xonly":
                    num_sb = small.tile(
                        [NCLS, H], fp32, name=f"num_sb_{m}", tag=f"num_{m}"
                    )
                    nc.vector.tensor_copy(out=num_sb, in_=pool_ps)
                    nc.gpsimd.dma_start(num_out[:, :], num_sb)

            def body():
                with nc.named_scope("tower_v"):
                    tower("v", VIR // 128, True, av_out, num_v)
                with nc.named_scope("tower_u"):
                    tower("u", EMB // 128, False, au_out, num_u)

            if reps == 1:
                body()
            else:
                # benchmarking variant: run the body `reps` times in a HW loop
                with tc.For_i(0, reps, 1):
                    body()

    nc.finalize()
    return nc


def _get_nc(reps=1, opts=None):
    key = ("nc", reps, tuple(sorted((opts or {}).items())))
    if key not in _CACHE:
        _CACHE[key] = _build_bass(reps, opts)
    return _CACHE[key]


def _prep_weights(params):
    def a32(x):
        return np.asarray(x, dtype=np.float32)

    def tile_bias(b, ntiles):
        return np.ascontiguousarray(a32(b).reshape(ntiles, 128).T)

    sh = {
        "wp16": a32(params["proj_v"]["w"]).astype(BF16),
        "bp": tile_bias(params["proj_v"]["b"], EMB // 128),
    }
    for m, att in (("v", params["attn_v"]), ("u", params["attn_u"])):
        sh[f"wfc_{m}16"] = a32(att["fc"]["w"]).astype(BF16)
        sh[f"bfc_{m}"] = tile_bias(att["fc"]["b"], H // 128)
        sh[f"wa_{m}16"] = a32(att["a"]["w"]).astype(BF16)
        sh[f"ba_{m}"] = tile_bias(att["a"]["b"], D // 128)
        sh[f"wb_{m}16"] = a32(att["b"]["w"]).astype(BF16)
        sh[f"bb_{m}"] = tile_bias(att["b"]["b"], D // 128)
        sh[f"wc_{m}16"] = a32(att["c"]["w"]).astype(BF16)
        sh[f"bc_{m}"] = np.ascontiguousarray(a32(att["c"]["b"]).reshape(NCLS, 1))
        sh[f"bcrow_{m}16"] = a32(att["c"]["b"]).reshape(1, NCLS).astype(BF16)
    return sh


def _run_device(h_virchow, h_UNI, params):
    from concourse.bass_utils import run_bass_kernel_spmd

    nc = _get_nc()
    shared = _prep_weights(params)
    in_maps = []
    for c in range(N_CORES):
        m = dict(shared)
        hv16 = np.zeros((NPAD, VIR), dtype=BF16)
        np.copyto(hv16[:NSH], h_virchow[c * NSH : (c + 1) * NSH], casting="unsafe")
        hu16 = np.zeros((NPAD, EMB), dtype=BF16)
        np.copyto(hu16[:NSH], h_UNI[c * NSH : (c + 1) * NSH], casting="unsafe")
        m["hv16"] = hv16
        m["hu16"] = hu16
        in_maps.append(m)

    res = run_bass_kernel_spmd(nc, in_maps, core_ids=list(range(N_CORES)))
    _CACHE["last_results"] = res
    return res.results


def _softmax(x, axis):
    x = x - np.max(x, axis=axis, keepdims=True)
    e = np.exp(x)
    return e / np.sum(e, axis=axis, keepdims=True)


def _host_tail(M_v, M_u, clinical, params):
    """Mirror of the reference MoE/gate/fusion/classifier tail in fp32 numpy."""

    def a32(x):
        return np.asarray(x, dtype=np.float32)

    def lin(p, x):
        return x @ a32(p["w"]) + a32(p["b"])

    def relu(x):
        return np.maximum(x, np.float32(0.0))

    def expert(p, x):
        return relu(lin(p["l2"], relu(lin(p["l1"], x))))

    c = relu(lin(params["clin"], a32(clinical)))
    g = np.concatenate([M_v.mean(0), M_u.mean(0)])
    gate_in = np.concatenate([g, c]).astype(np.float32)
    gate_logits = lin(
        params["gate"]["l2"], relu(lin(params["gate"]["l1"], gate_in))
    )[None, :]

    gum = -np.log(-np.log(GUMBEL_U + np.float32(EPS)) + np.float32(EPS)).astype(
        np.float32
    )
    y = _softmax((gate_logits + gum) / np.float32(TAU), axis=-1).astype(np.float32)
    idx = int(np.argmax(y, axis=-1)[0])
    y_hard = np.zeros_like(y)
    y_hard[0, idx] = 1.0
    w = ((y_hard - y) + y)[0]

    E1 = expert(params["e1"], M_v)
    E3 = expert(params["e3"], M_u)
    E2 = lin(
        params["e2_out"],
        expert(params["e2"], np.concatenate([M_v, M_u], axis=-1)),
    )
    M_moe = w[0] * E1 + w[1] * E2 + w[2] * E3

    c_b = np.broadcast_to(c, M_moe.shape)
    M_fused = lin(params["fusion"], np.concatenate([M_moe, c_b], axis=-1))

    cls_w = a32(params["cls"]["w"])
    cls_b = a32(params["cls"]["b"])
    logits = (np.sum(M_fused * cls_w, axis=-1) + cls_b)[None, :].astype(np.float32)
    Y_prob = _softmax(logits, axis=1).astype(np.float32)
    Y_hat = np.argmax(logits, axis=1).astype(np.int32)
    return logits, Y_prob, Y_hat


def kernel(h_virchow, h_UNI, clinical, params):
    h_virchow = np.asarray(h_virchow, dtype=np.float32)
    h_UNI = np.asarray(h_UNI, dtype=np.float32)
    clinical = np.asarray(clinical, dtype=np.float32)

    results = _run_device(h_virchow, h_UNI, params)

    A_v = np.concatenate([r["av_out"] for r in results], axis=1)
    A_u = np.concatenate([r["au_out"] for r in results], axis=1)

    def pooled(a_full, key):
        num = np.zeros((NCLS, H), dtype=np.float64)
        for r in results:
            num += r[key].astype(np.float64)
        Z = np.exp(a_full.astype(np.float64)).sum(axis=1)
        return (num / Z[:, None]).astype(np.float32)

    M_v = pooled(A_v, "num_v")
    M_u = pooled(A_u, "num_u")

    logits, Y_prob, Y_hat = _host_tail(M_v, M_u, clinical, params)
    return logits, Y_prob, Y_hat, A_v, A_u


# revision 26
# speedup vs baseline: 6.9856x; 5.0187x over previous
"""Trainium2 Bass kernel for nn_CaPa_MoE_clinical_MLP (CLAM-style dual-tower
attention-MIL with MoE head).

Strategy (8 NeuronCores, SPMD, no collectives):
  - Shard the N=30000 patch dimension: 3750 rows per core (padded to 3840
    on-device).
  - Per core: project h_virchow (2560->1024), attention nets for both
    modalities, raw attention scores A_raw [2, n_local] (kernel output), and
    the softmax-numerator partial  num = exp(A_raw).T @ hp  [2, 512] plus
    nothing else -- exp() without max-shift is safe because |A_raw| <~ 4.
  - Host: combine per-core partials (Z from the full A_raw output), then run
    the tiny expert/gate/fusion/classifier tail in fp32 numpy.

Matmuls run in bf16 with fp32 PSUM accumulation.  Layout trick: activations
are kept feature-major ("transposed", [f, n]); using a transposed activation
tile as matmul lhsT yields natural-layout outputs and vice versa, so only the
raw inputs need a real transpose, done via HWDGE xbar DMA-transpose (2-byte
dtype) from a bf16 DRAM staging copy produced by a casting SWDGE DMA.
"""

import numpy as np
import ml_dtypes

# ---- problem constants (hardcoded per task contract) ----
N_CORES = 8
N_TOTAL = 30000
NSH = N_TOTAL // N_CORES        # 3750 rows per core
NPAD = 3840                     # 30 * 128
VIR, EMB, H, D, NCLS, CLIN = 2560, 1024, 512, 256, 2, 6
TAU, EPS = 1.0, 1e-20

# jax.random.uniform(jax.random.key(42), (1, 3)) -- fixed in the reference.
# Filled in from the cached reference; verified by test.py.
GUMBEL_U = np.array([[0.59400654, 0.43801308, 0.6285691]], dtype=np.float32)

BF16 = ml_dtypes.bfloat16

_CACHE = {}


def _blocks():
    """(n0, nb, valid) DMA/compute blocks covering NPAD rows."""
    out = []
    n0 = 0
    while n0 < NPAD:
        nb = min(512, NPAD - n0)
        out.append((n0, nb, max(0, min(NSH - n0, nb))))
        n0 += nb
    return out


def _build_bass(reps=1, opts=None):
    import concourse.mybir as mybir
    from concourse import bacc
    from concourse.tile import TileContext
    from concourse.masks import make_identity

    fp32 = mybir.dt.float32
    bf16 = mybir.dt.bfloat16
    AF = mybir.ActivationFunctionType

    opts = dict(opts or {})
    xin_bufs = opts.get("xin_bufs", 3)
    work_bufs = opts.get("work_bufs", 2)
    split_xpose = opts.get("split_xpose", False)   # alternate SP/ACT rings
    ablate = opts.get("ablate", None)              # None | "noxpose" | "xonly"

    nc = bacc.Bacc("TRN2", target_bir_lowering=False)

    # ---------------- I/O ----------------
    # bf16, host-padded to NPAD rows (zeros) so xbar transpose tiling is clean
    hv = nc.dram_tensor("hv16", [NPAD, VIR], bf16, kind="ExternalInput")
    hu = nc.dram_tensor("hu16", [NPAD, EMB], bf16, kind="ExternalInput")

    def win(name, shape, dt=bf16):
        return nc.dram_tensor(name, shape, dt, kind="ExternalInput")

    wp = win("wp16", [VIR, EMB])
    bp = win("bp", [128, EMB // 128], fp32)
    wio = {}
    for m in ("v", "u"):
        wio[f"wfc_{m}"] = win(f"wfc_{m}16", [EMB, H])
        wio[f"bfc_{m}"] = win(f"bfc_{m}", [128, H // 128], fp32)
        wio[f"wa_{m}"] = win(f"wa_{m}16", [H, D])
        wio[f"ba_{m}"] = win(f"ba_{m}", [128, D // 128], fp32)
        wio[f"wb_{m}"] = win(f"wb_{m}16", [H, D])
        wio[f"bb_{m}"] = win(f"bb_{m}", [128, D // 128], fp32)
        wio[f"wc_{m}"] = win(f"wc_{m}16", [D, NCLS])
        wio[f"bc_{m}"] = win(f"bc_{m}", [NCLS, 1], fp32)
        wio[f"bcrow_{m}"] = win(f"bcrow_{m}16", [1, NCLS])

    av_out = nc.dram_tensor("av_out", [NCLS, NSH], fp32, kind="ExternalOutput")
    au_out = nc.dram_tensor("au_out", [NCLS, NSH], fp32, kind="ExternalOutput")
    num_v = nc.dram_tensor("num_v", [NCLS, H], fp32, kind="ExternalOutput")
    num_u = nc.dram_tensor("num_u", [NCLS, H], fp32, kind="ExternalOutput")

    blocks = _blocks()

    with TileContext(nc) as tc:
        with (
            tc.tile_pool(name="consts", bufs=1) as consts,
            tc.tile_pool(name="wpool", bufs=1) as wpool,
            tc.tile_pool(name="xin", bufs=xin_bufs) as xinp,
            tc.tile_pool(name="work", bufs=work_bufs) as work,
            tc.tile_pool(name="hpn", bufs=4) as hpnp,
            tc.tile_pool(name="small", bufs=1) as small,
            tc.tile_pool(name="mmps", bufs=3, space="PSUM") as mmps,
            tc.tile_pool(name="tpps", bufs=2, space="PSUM") as tpps,
            tc.tile_pool(name="pnps", bufs=1, space="PSUM") as pnps,
            tc.tile_pool(name="plps", bufs=1, space="PSUM") as plps,
        ):
            # ---------------- constants / weights to SBUF ----------------
            ident = consts.tile([128, 128], bf16)
            make_identity(nc, ident)
            ones_row = consts.tile([1, 128], bf16)
            nc.vector.memset(ones_row, 1.0)

            wp_sb = wpool.tile([128, VIR // 128, EMB], bf16)
            nc.gpsimd.dma_start(wp_sb, wp[:, :].rearrange("(kt p) f -> p kt f", p=128))
            bp_sb = wpool.tile([128, EMB // 128], fp32)
            nc.gpsimd.dma_start(bp_sb, bp[:, :])

            wsb = {}
            for m in ("v", "u"):
                for key, kdim, fdim in (
                    (f"wfc_{m}", EMB, H),
                    (f"wa_{m}", H, D),
                    (f"wb_{m}", H, D),
                    (f"wc_{m}", D, NCLS),
                ):
                    t = wpool.tile([128, kdim // 128, fdim], bf16, name=f"sb_{key}")
                    nc.gpsimd.dma_start(
                        t, wio[key][:, :].rearrange("(kt p) f -> p kt f", p=128)
                    )
                    wsb[key] = t
                for key in (f"bfc_{m}", f"ba_{m}", f"bb_{m}", f"bc_{m}"):
                    t = wpool.tile(list(wio[key].shape), fp32, name=f"sb_{key}")
                    nc.gpsimd.dma_start(t, wio[key][:, :])
                    wsb[key] = t
                t = wpool.tile([1, NCLS], bf16, name=f"sb_bcrow_{m}")
                nc.gpsimd.dma_start(t, wio[f"bcrow_{m}"][:, :])
                wsb[f"bcrow_{m}"] = t

            h16 = {"v": hv, "u": hu}

            # ---------------- per-modality tower ----------------
            def tower(m, kin_tiles, has_proj, a_out, num_out):
                w_fc = wsb[f"wfc_{m}"]
                b_fc = wsb[f"bfc_{m}"]
                w_a, b_a = wsb[f"wa_{m}"], wsb[f"ba_{m}"]
                w_b, b_b = wsb[f"wb_{m}"], wsb[f"bb_{m}"]
                w_c, b_c = wsb[f"wc_{m}"], wsb[f"bc_{m}"]
                bcrow = wsb[f"bcrow_{m}"]

                pool_ps = plps.tile([NCLS, H], fp32, name=f"pool_{m}", tag=f"pool_{m}")
                n_pool_mm = sum(nb // 128 for _, nb, _ in blocks)
                pool_i = 0

                for b, (n0, nb, valid) in enumerate(blocks):
                    src = h16[m]
                    # transposed input tiles: one 3D xbar-transpose DMA per
                    # block, [nb, kin*128] -> [128, kin, nb]
                    xt = xinp.tile(
                        [128, kin_tiles, nb], bf16, name=f"xin_{m}{b}", tag="xin"
                    )
                    if ablate == "noxpose":
                        nc.vector.memset(xt, 0.001)
                    else:
                        eng = nc.scalar if (split_xpose and b % 2) else nc.sync
                        eng.dma_start_transpose(
                            xt,
                            src[n0 : n0 + nb, :].rearrange(
                                "n (kt p) -> n kt p", p=128
                            ),
                        )
                    xin = [xt[:, kt, :] for kt in range(kin_tiles)]
                    if ablate == "xonly":
                        continue

                    if has_proj:
                        x16 = work.tile(
                            [128, EMB // 128, nb], bf16, name=f"x16_{b}", tag="x16"
                        )
                        for ft in range(EMB // 128):
                            ps = mmps.tile(
                                [128, 512], fp32, name=f"ps_p{b}_{ft}", tag="mm"
                            )[:, :nb]
                            for kt in range(kin_tiles):
                                nc.tensor.matmul(
                                    ps,
                                    wp_sb[:, kt, ft * 128 : (ft + 1) * 128],
                                    xin[kt],
                                    start=(kt == 0),
                                    stop=(kt == kin_tiles - 1),
                                )
                            nc.scalar.activation(
                                x16[:, ft, :], ps, AF.Identity,
                                bias=bp_sb[:, ft : ft + 1],
                            )
                        feat = [x16[:, j, :] for j in range(EMB // 128)]
                    else:
                        feat = xin

                    nfeat = len(feat)
                    # fc -> hpt [f', n] (relu, transposed layout)
                    hpt = work.tile(
                        [128, H // 128, nb], bf16, name=f"hpt_{m}{b}", tag="hpt"
                    )
                    for ft in range(H // 128):
                        ps = mmps.tile(
                            [128, 512], fp32, name=f"ps_fc{b}_{ft}", tag="mm"
                        )[:, :nb]
                        for kt in range(nfeat):
                            nc.tensor.matmul(
                                ps,
                                w_fc[:, kt, ft * 128 : (ft + 1) * 128],
                                feat[kt],
                                start=(kt == 0),
                                stop=(kt == nfeat - 1),
                            )
                        nc.scalar.activation(
                            hpt[:, ft, :], ps, AF.Relu, bias=b_fc[:, ft : ft + 1]
                        )

                    # gated attention: aT = tanh(.), bT = sigmoid(.), gT = aT*bT
                    gt = work.tile(
                        [128, D // 128, nb], bf16, name=f"gt_{m}{b}", tag="gt"
                    )
                    at = work.tile(
                        [128, D // 128, nb], bf16, name=f"at_{m}{b}", tag="at"
                    )
                    bt = work.tile(
                        [128, D // 128, nb], bf16, name=f"bt_{m}{b}", tag="bt"
                    )
                    for ft in range(D // 128):
                        psa = mmps.tile(
                            [128, 512], fp32, name=f"ps_a{b}_{ft}", tag="mm"
                        )[:, :nb]
                        for kt in range(H // 128):
                            nc.tensor.matmul(
                                psa,
                                w_a[:, kt, ft * 128 : (ft + 1) * 128],
                                hpt[:, kt, :],
                                start=(kt == 0),
                                stop=(kt == H // 128 - 1),
                            )
                        nc.scalar.activation(
                            at[:, ft, :], psa, AF.Tanh, bias=b_a[:, ft : ft + 1]
                        )
                        psb = mmps.tile(
                            [128, 512], fp32, name=f"ps_b{b}_{ft}", tag="mm"
                        )[:, :nb]
                        for kt in range(H // 128):
                            nc.tensor.matmul(
                                psb,
                                w_b[:, kt, ft * 128 : (ft + 1) * 128],
                                hpt[:, kt, :],
                                start=(kt == 0),
                                stop=(kt == H // 128 - 1),
                            )
                        nc.scalar.activation(
                            bt[:, ft, :], psb, AF.Sigmoid, bias=b_b[:, ft : ft + 1]
                        )
                        nc.vector.tensor_mul(
                            out=gt[:, ft, :], in0=at[:, ft, :], in1=bt[:, ft, :]
                        )

                    # raw attention scores, transposed layout [2, nb] (output)
                    psr = mmps.tile([128, 512], fp32, name=f"ps_r{b}", tag="mm")[
                        :NCLS, :nb
                    ]
                    for kt in range(D // 128):
                        nc.tensor.matmul(
                            psr,
                            w_c[:, kt, :],
                            gt[:, kt, :],
                            start=(kt == 0),
                            stop=(kt == D // 128 - 1),
                        )
                    if valid > 0:
                        araw = work.tile(
                            [NCLS, 512], fp32, name=f"araw_{m}{b}", tag="araw"
                        )[:, :nb]
                        nc.scalar.activation(araw, psr, AF.Identity, bias=b_c)
                        nc.gpsimd.dma_start(
                            a_out[:, n0 : n0 + valid], araw[:, :valid]
                        )

                    # pooling: per 128-row subtile, natural-layout exp weights
                    # via matmul (lhsT = transposed activations trick), then
                    # num += E_nat.T @ HP_nat
                    for s in range(nb // 128):
                        sl = slice(s * 128, (s + 1) * 128)
                        # araw in natural layout [128n, 2]
                        psn = pnps.tile(
                            [128, NCLS], fp32, name=f"psn_{m}{b}_{s}", tag="pnat"
                        )
                        for kt in range(D // 128):
                            nc.tensor.matmul(
                                psn,
                                gt[:, kt, sl],
                                w_c[:, kt, :],
                                start=(kt == 0),
                                stop=False,
                            )
                        # fold per-class bias: += ones.T @ bcrow
                        nc.tensor.matmul(
                            psn, ones_row, bcrow, start=False, stop=True
                        )
                        e_nat = hpnp.tile(
                            [128, NCLS], bf16, name=f"enat_{m}{b}_{s}", tag="enat"
                        )
                        sub_valid = max(0, min(valid - s * 128, 128))
                        if sub_valid < 128:
                            # zero first, then exp() only the valid rows
                            # (partition offsets must be 32-aligned, so we
                            # cannot memset [sub_valid:, :] directly)
                            nc.vector.memset(e_nat, 0.0)
                            nc.scalar.activation(
                                e_nat[:sub_valid, :], psn[:sub_valid, :], AF.Exp
                            )
                        else:
                            nc.scalar.activation(e_nat, psn, AF.Exp)

                        # HP natural layout via PE transpose of hpt
                        hpn = hpnp.tile(
                            [128, H], bf16, name=f"hpn_{m}{b}_{s}", tag="hpn"
                        )
                        for ft in range(H // 128):
                            pst = tpps.tile(
                                [128, 128], bf16, name=f"pst_{m}{b}_{s}_{ft}",
                                tag="tp",
                            )
                            nc.tensor.transpose(pst, hpt[:, ft, sl], ident)
                            nc.vector.tensor_copy(
                                out=hpn[:, ft * 128 : (ft + 1) * 128], in_=pst
                            )

                        nc.tensor.matmul(
                            pool_ps,
                            e_nat,
                            hpn,
                            start=(pool_i == 0),
                            stop=(pool_i == n_pool_mm - 1),
                        )
                        pool_i += 1

                if ablate != "xonly":
                    num_sb = small.tile(
                        [NCLS, H], fp32, name=f"num_sb_{m}", tag=f"num_{m}"
                    )
                    nc.vector.tensor_copy(out=num_sb, in_=pool_ps)
                    nc.gpsimd.dma_start(num_out[:, :], num_sb)

            def body():
                with nc.named_scope("tower_v"):
                    tower("v", VIR // 128, True, av_out, num_v)
                with nc.named_scope("tower_u"):
                    tower("u", EMB // 128, False, au_out, num_u)

            if reps == 1:
                body()
            else:
                # benchmarking variant: run the body `reps` times in a HW loop
                with tc.For_i(0, reps, 1):
                    body()

    nc.finalize()
    return nc


def _get_nc(reps=1, opts=None):
    key = ("nc", reps, tuple(sorted((opts or {}).items())))
    if key not in _CACHE:
        _CACHE[key] = _build_bass(reps, opts)
    return _CACHE[key]


def _prep_weights(params):
    def a32(x):
        return np.asarray(x, dtype=np.float32)

    def tile_bias(b, ntiles):
        return np.ascontiguousarray(a32(b).reshape(ntiles, 128).T)

    sh = {
        "wp16": a32(params["proj_v"]["w"]).astype(BF16),
        "bp": tile_bias(params["proj_v"]["b"], EMB // 128),
    }
    for m, att in (("v", params["attn_v"]), ("u", params["attn_u"])):
        sh[f"wfc_{m}16"] = a32(att["fc"]["w"]).astype(BF16)
        sh[f"bfc_{m}"] = tile_bias(att["fc"]["b"], H // 128)
        sh[f"wa_{m}16"] = a32(att["a"]["w"]).astype(BF16)
        sh[f"ba_{m}"] = tile_bias(att["a"]["b"], D // 128)
        sh[f"wb_{m}16"] = a32(att["b"]["w"]).astype(BF16)
        sh[f"bb_{m}"] = tile_bias(att["b"]["b"], D // 128)
        sh[f"wc_{m}16"] = a32(att["c"]["w"]).astype(BF16)
        sh[f"bc_{m}"] = np.ascontiguousarray(a32(att["c"]["b"]).reshape(NCLS, 1))
        sh[f"bcrow_{m}16"] = a32(att["c"]["b"]).reshape(1, NCLS).astype(BF16)
    return sh


def _run_device(h_virchow, h_UNI, params):
    from concourse.bass_utils import run_bass_kernel_spmd

    nc = _get_nc()
    shared = _prep_weights(params)
    in_maps = []
    for c in range(N_CORES):
        m = dict(shared)
        hv16 = np.zeros((NPAD, VIR), dtype=BF16)
        np.copyto(hv16[:NSH], h_virchow[c * NSH : (c + 1) * NSH], casting="unsafe")
        hu16 = np.zeros((NPAD, EMB), dtype=BF16)
        np.copyto(hu16[:NSH], h_UNI[c * NSH : (c + 1) * NSH], casting="unsafe")
        m["hv16"] = hv16
        m["hu16"] = hu16
        in_maps.append(m)

    res = run_bass_kernel_spmd(nc, in_maps, core_ids=list(range(N_CORES)))
    _CACHE["last_results"] = res
    return res.results


def _softmax(x, axis):
    x = x - np.max(x, axis=axis, keepdims=True)
    e = np.exp(x)
    return e / np.sum(e, axis=axis, keepdims=True)


def _host_tail(M_v, M_u, clinical, params):
    """Mirror of the reference MoE/gate/fusion/classifier tail in fp32 numpy."""

    def a32(x):
        return np.asarray(x, dtype=np.float32)

    def lin(p, x):
        return x @ a32(p["w"]) + a32(p["b"])

    def relu(x):
        return np.maximum(x, np.float32(0.0))

    def expert(p, x):
        return relu(lin(p["l2"], relu(lin(p["l1"], x))))

    c = relu(lin(params["clin"], a32(clinical)))
    g = np.concatenate([M_v.mean(0), M_u.mean(0)])
    gate_in = np.concatenate([g, c]).astype(np.float32)
    gate_logits = lin(
        params["gate"]["l2"], relu(lin(params["gate"]["l1"], gate_in))
    )[None, :]

    gum = -np.log(-np.log(GUMBEL_U + np.float32(EPS)) + np.float32(EPS)).astype(
        np.float32
    )
    y = _softmax((gate_logits + gum) / np.float32(TAU), axis=-1).astype(np.float32)
    idx = int(np.argmax(y, axis=-1)[0])
    y_hard = np.zeros_like(y)
    y_hard[0, idx] = 1.0
    w = ((y_hard - y) + y)[0]

    E1 = expert(params["e1"], M_v)
    E3 = expert(params["e3"], M_u)
    E2 = lin(
        params["e2_out"],
        expert(params["e2"], np.concatenate([M_v, M_u], axis=-1)),
    )
    M_moe = w[0] * E1 + w[1] * E2 + w[2] * E3

    c_b = np.broadcast_to(c, M_moe.shape)
    M_fused = lin(params["fusion"], np.concatenate([M_moe, c_b], axis=-1))

    cls_w = a32(params["cls"]["w"])
    cls_b = a32(params["cls"]["b"])
    logits = (np.sum(M_fused * cls_w, axis=-1) + cls_b)[None, :].astype(np.float32)
    Y_prob = _softmax(logits, axis=1).astype(np.float32)
    Y_hat = np.argmax(logits, axis=1).astype(np.int32)
    return logits, Y_prob, Y_hat


def kernel(h_virchow, h_UNI, clinical, params):
    h_virchow = np.asarray(h_virchow, dtype=np.float32)
    h_UNI = np.asarray(h_UNI, dtype=np.float32)
    clinical = np.asarray(clinical, dtype=np.float32)

    results = _run_device(h_virchow, h_UNI, params)

    A_v = np.concatenate([r["av_out"] for r in results], axis=1)
    A_u = np.concatenate([r["au_out"] for r in results], axis=1)

    def pooled(a_full, key):
        num = np.zeros((NCLS, H), dtype=np.float64)
        for r in results:
            num += r[key].astype(np.float64)
        Z = np.exp(a_full.astype(np.float64)).sum(axis=1)
        return (num / Z[:, None]).astype(np.float32)

    M_v = pooled(A_v, "num_v")
    M_u = pooled(A_u, "num_u")

    logits, Y_prob, Y_hat = _host_tail(M_v, M_u, clinical, params)
    return logits, Y_prob, Y_hat, A_v, A_u
